# revision 1
# baseline (speedup 1.0000x reference)
"""Trainium2 Bass kernel for nn_RecPolicy (7-joint up/down GRU policy net).

Data-parallel over 8 NeuronCores: each core runs batch 131072, tiled as
2 pairs x 2 superchunks x 64 groups x 512 columns. The tiny [2->6] GRU
linear maps are expanded on the host into 128x128 block-diagonal (kron
with I_64) f16 matrices so one matmul processes 64 batch groups; gate
tensors live as [comp*64g, cols] tiles so ACT/DVE ops run at full 128
partitions. PSUM accumulation absorbs the n-gate add (ghn*r + gin); the
h-update is 3 f16 tensor ops. Host: x -> xT f16 per core; y = yT.T + out_b.
"""
import os
import sys

import numpy as np

for _p in ("/opt/trn_rl_repo", "/root/.axon_site/_ro/trn_rl_repo"):
    if os.path.isdir(_p) and _p not in sys.path:
        sys.path.insert(0, _p)

B = 1048576
NCORES = 8
BC = B // NCORES          # 131072 per core
G = 64                    # batch groups packed per matmul
N = 512                   # moving free dim (columns) per matmul
S = BC // (G * N)         # 4 superchunks
Q = S // 2                # 2 pairs, each = 2 superchunks side by side
W = 2 * N                 # 1024: pair-wide free dim

# tuning flags (sim-swept)
CFG = {
    "wide_sig": False,    # sigmoid over [128, W] paired psum (bufs=1) vs per-s
    "wide_n": False,      # STT+tanh over paired pn psum
    "wide_h": False,      # D/E/H' as wide [128, W] ops (h tiles are always wide)
    "d_on_pool": False,   # D = h - n on GPSIMD
}

_CACHE = {}


def _build_bass(cfg=CFG):
    import concourse.bass as bass
    import concourse.bacc as bacc
    import concourse.mybir as mybir
    from concourse.tile import TileContext

    dt = mybir.dt
    AF = mybir.ActivationFunctionType
    ALU = mybir.AluOpType

    nc = bacc.Bacc("TRN2", target_bir_lowering=False)

    xT = nc.dram_tensor("xT", [19, BC], dt.float16, kind="ExternalInput")
    yT = nc.dram_tensor("yT", [7, BC], dt.float32, kind="ExternalOutput")

    lw_shapes = {}
    for pre in ("up", "dn"):
        for part in ("x_r", "x_z", "x_n", "h_r", "h_z", "h_n"):
            lw_shapes[f"{pre}_{part}"] = [2 * G, 2 * G]
    lw_shapes["obs01"] = [2 * G, 2 * G]
    lw_shapes["obs23"] = [2 * G, 2 * G]
    lw_shapes["obs4"] = [G, 2 * G]
    lw_shapes["obsh"] = [2 * G, 2 * G]
    lw_shapes["out"] = [2 * G, G]
    lw_order = list(lw_shapes)
    lwcat_dram = nc.dram_tensor(
        "lwcat", [2 * G, 2 * G * len(lw_order)], dt.float16, kind="ExternalInput"
    )

    bias_names = [
        "up_r", "up_z", "up_bhhn", "up_bihn",
        "dn_r", "dn_z", "dn_bhhn", "dn_bihn", "obs",
    ]
    biascat_dram = nc.dram_tensor(
        "biascat", [2 * G, len(bias_names)], dt.float32, kind="ExternalInput"
    )

    # xTv[f, q] is [g, m]: batch b = q*2GN + g*W + m, m in [0, W)
    xTv = xT.rearrange("f (q g m) -> f q g m", q=Q, g=G, m=W)
    # yTw[t, q] is [g, m]
    yTw = yT.rearrange("t (q g m) -> t q g m", q=Q, g=G, m=W)

    with TileContext(nc) as tc:
        with (
            tc.tile_pool(name="const", bufs=1) as cpool,
            tc.tile_pool(name="persist", bufs=1) as hpool,
            tc.tile_pool(name="xin", bufs=4) as xpool,
            tc.tile_pool(name="gates", bufs=4) as spool,
            tc.tile_pool(name="tmps", bufs=4) as tpool,
            tc.tile_pool(name="outs", bufs=2) as opool,
            tc.tile_pool(name="psum", bufs=1, space="PSUM") as ppool,
        ):
            lwcat = cpool.tile([2 * G, 2 * G * len(lw_order)], dt.float16, tag="lwcat", name="lwcat")
            nc.sync.dma_start(out=lwcat[:], in_=lwcat_dram[:])
            lw = {}
            for i, k in enumerate(lw_order):
                kk, mm = lw_shapes[k]
                lw[k] = lwcat[0:kk, i * 2 * G: i * 2 * G + mm]
            biascat = cpool.tile([2 * G, len(bias_names)], dt.float32, tag="biascat", name="biascat")
            nc.sync.dma_start(out=biascat[:], in_=biascat_dram[:])
            bias = {k: biascat[:, i:i + 1] for i, k in enumerate(bias_names)}

            h_up = {}   # (t, q) -> wide tile [128, W]
            h_dn = {}   # (q, parity)
            h0_dn = {}  # q
            for q in range(Q):
                for t in range(7):
                    h_up[(t, q)] = hpool.tile([2 * G, W], dt.float16, tag=f"hup_{t}_{q}", name=f"hup_{t}_{q}")
                for p in range(2):
                    h_dn[(q, p)] = hpool.tile([2 * G, W], dt.float16, tag=f"hdn_{q}_{p}", name=f"hdn_{q}_{p}")
                h0_dn[q] = hpool.tile([2 * G, W], dt.float16, tag=f"h0dn_{q}", name=f"h0dn_{q}")

            def cols(si):
                return slice(si * N, (si + 1) * N)

            # PSUM tiles. bufs set so total fits in 8 banks (per-bank = [128, 512] f32).
            # narrow mode: pr/pz/pn [128,512] bufs=2 -> 6 banks; pact [128,W] bufs=1 -> 2. = 8
            # wide_sig: pr/pz [128,W] bufs=1 -> 4 banks; pn narrow bufs=2 -> 2; pact -> 2. = 8
            # wide_sig+wide_n: pr/pz/pn wide bufs=1 -> 6; pact -> 2. = 8
            def psum_rz():
                if cfg["wide_sig"]:
                    pr = ppool.tile([2 * G, W], dt.float32, tag="pr", name="pr")
                    pz = ppool.tile([2 * G, W], dt.float32, tag="pz", name="pz")
                    return [(pr, slice(0, W))], [(pz, slice(0, W))]
                prs = [(ppool.tile([2 * G, N], dt.float32, tag="pr", bufs=2, name="pr"), cols(si)) for si in range(2)]
                pzs = [(ppool.tile([2 * G, N], dt.float32, tag="pz", bufs=2, name="pz"), cols(si)) for si in range(2)]
                return prs, pzs

            def psum_n():
                if cfg["wide_n"]:
                    return [(ppool.tile([2 * G, W], dt.float32, tag="pn", name="pn"), slice(0, W))]
                return [(ppool.tile([2 * G, N], dt.float32, tag="pn", bufs=2, name="pn"), cols(si)) for si in range(2)]

            def gru_step(pre, q, x_in, h_prev, h_out, first):
                """x_in, h_prev, h_out: [128, W] f16 wide tiles (h_prev None if zero)."""
                prs, pzs = psum_rz()
                for pp, cc in prs:
                    for si in range(2):
                        c = cols(si)
                        if c.start < cc.start or c.stop > cc.stop:
                            continue
                        lc = slice(c.start - cc.start, c.stop - cc.start)
                        nc.tensor.matmul(pp[:, lc], lw[pre + "_x_r"][:], x_in[:, c], start=True, stop=first)
                        if not first:
                            nc.tensor.matmul(pp[:, lc], lw[pre + "_h_r"][:], h_prev[:, c], start=False, stop=True)
                for pp, cc in pzs:
                    for si in range(2):
                        c = cols(si)
                        if c.start < cc.start or c.stop > cc.stop:
                            continue
                        lc = slice(c.start - cc.start, c.stop - cc.start)
                        nc.tensor.matmul(pp[:, lc], lw[pre + "_x_z"][:], x_in[:, c], start=True, stop=first)
                        if not first:
                            nc.tensor.matmul(pp[:, lc], lw[pre + "_h_z"][:], h_prev[:, c], start=False, stop=True)
                R = spool.tile([2 * G, W], dt.float16, tag="R", name="R")
                Z = spool.tile([2 * G, W], dt.float16, tag="Z", name="Z")
                for pp, cc in prs:
                    nc.scalar.activation(R[:, cc], pp[:], AF.Sigmoid, bias=bias[pre + "_r"][:])
                for pp, cc in pzs:
                    nc.scalar.activation(Z[:, cc], pp[:], AF.Sigmoid, bias=bias[pre + "_z"][:])
                NT = spool.tile([2 * G, W], dt.float16, tag="NT", name="NT")
                for pp, cc in psum_n():
                    sis = [si for si in range(2) if cols(si).start >= cc.start and cols(si).stop <= cc.stop]
                    if first:
                        for si in sis:
                            c = cols(si)
                            lc = slice(c.start - cc.start, c.stop - cc.start)
                            nc.tensor.matmul(pp[:, lc], lw[pre + "_x_n"][:], x_in[:, c], start=True, stop=True)
                        nc.vector.scalar_tensor_tensor(
                            out=pp[:], in0=R[:, cc], scalar=bias[pre + "_bhhn"][:], in1=pp[:],
                            op0=ALU.mult, op1=ALU.add,
                        )
                    else:
                        for si in sis:
                            c = cols(si)
                            lc = slice(c.start - cc.start, c.stop - cc.start)
                            nc.tensor.matmul(pp[:, lc], lw[pre + "_h_n"][:], h_prev[:, c], start=True, stop=False)
                        nc.vector.scalar_tensor_tensor(
                            out=pp[:], in0=pp[:], scalar=bias[pre + "_bhhn"][:], in1=R[:, cc],
                            op0=ALU.add, op1=ALU.mult,
                        )
                        for si in sis:
                            c = cols(si)
                            lc = slice(c.start - cc.start, c.stop - cc.start)
                            nc.tensor.matmul(
                                pp[:, lc], lw[pre + "_x_n"][:], x_in[:, c], start=False, stop=True,
                                skip_group_check=True,
                            )
                    nc.scalar.activation(NT[:, cc], pp[:], AF.Tanh, bias=bias[pre + "_bihn"][:])
                # h' = n + z * (h_prev - n)
                hcols = [slice(0, W)] if cfg["wide_h"] else [cols(0), cols(1)]
                for hc in hcols:
                    E = tpool.tile([2 * G, W], dt.float16, tag="E", name="E", bufs=4)
                    if first:
                        nc.vector.tensor_mul(out=E[:, hc], in0=Z[:, hc], in1=NT[:, hc])
                        nc.vector.tensor_sub(out=h_out[:, hc], in0=NT[:, hc], in1=E[:, hc])
                    else:
                        D = tpool.tile([2 * G, W], dt.float16, tag="D", name="D", bufs=4)
                        eng = nc.gpsimd if cfg["d_on_pool"] else nc.vector
                        eng.tensor_sub(out=D[:, hc], in0=h_prev[:, hc], in1=NT[:, hc])
                        nc.vector.tensor_mul(out=E[:, hc], in0=Z[:, hc], in1=D[:, hc])
                        nc.vector.tensor_add(out=h_out[:, hc], in0=NT[:, hc], in1=E[:, hc])

            def load_xpair(f0, f1, q, tag):
                t = xpool.tile([2 * G, W], dt.float16, tag=tag, name="xtile")
                nc.sync.dma_start(out=t[0:G, :], in_=xTv[f0, q])
                nc.sync.dma_start(out=t[G:2 * G, :], in_=xTv[f1, q])
                return t

            # ---- up pass ----
            for t in range(7):
                for q in range(Q):
                    xr = load_xpair(5 + t, 12 + t, q, "xr")
                    h_prev = None if t == 0 else h_up[(t - 1, q)]
                    gru_step("up", q, xr, h_prev, h_up[(t, q)], first=(t == 0))

            # ---- obs mix ----
            for q in range(Q):
                o01 = load_xpair(0, 1, q, "xr")
                o23 = load_xpair(2, 3, q, "xr")
                o4 = xpool.tile([G, W], dt.float16, tag="o4", name="o4")
                nc.sync.dma_start(out=o4[:], in_=xTv[4, q])
                for pp, cc in psum_n():
                    for si in range(2):
                        c = cols(si)
                        if c.start < cc.start or c.stop > cc.stop:
                            continue
                        lc = slice(c.start - cc.start, c.stop - cc.start)
                        nc.tensor.matmul(pp[:, lc], lw["obs01"][:], o01[:, c], start=True, stop=False)
                        nc.tensor.matmul(pp[:, lc], lw["obs23"][:], o23[:, c], start=False, stop=False)
                        nc.tensor.matmul(pp[:, lc], lw["obs4"][:], o4[:, c], start=False, stop=False)
                        nc.tensor.matmul(pp[:, lc], lw["obsh"][:], h_up[(6, q)][:, c], start=False, stop=True)
                    nc.vector.tensor_scalar_add(out=h0_dn[q][:, cc], in0=pp[:], scalar1=bias["obs"][:])

            # ---- down pass ----
            for t in range(7):
                pact = ppool.tile([2 * G, W], dt.float32, tag="pact", name="pact")
                for q in range(Q):
                    h_prev = h0_dn[q] if t == 0 else h_dn[(q, (t - 1) % 2)]
                    h_new = h_dn[(q, t % 2)]
                    gru_step("dn", q, h_up[(t, q)], h_prev, h_new, first=False)
                    rows = slice(q * G, (q + 1) * G)
                    for si in range(2):
                        c = cols(si)
                        nc.tensor.matmul(pact[rows, c], lw["out"][:], h_new[:, c], start=True, stop=True)
                oact = opool.tile([2 * G, W], dt.float32, tag="oact", name="oact")
                nc.vector.tensor_copy(out=oact[:], in_=pact[:])
                for q in range(Q):
                    nc.gpsimd.dma_start(out=yTw[t, q], in_=oact[q * G:(q + 1) * G, :])

    nc.compile()
    return nc


def _prepare_shared(inputs):
    f16 = np.float16
    f32 = np.float32
    I = np.eye(G, dtype=f32)

    def kron16(a):
        return np.kron(np.asarray(a, f32), I).astype(f16)

    def pcol(v):
        return np.ascontiguousarray(
            np.repeat(np.asarray(v, f32).reshape(-1), G)[:, None]
        )

    up_wih = np.asarray(inputs["up_wih"], f32)
    up_whh = np.asarray(inputs["up_whh"], f32)
    dn_wih = np.asarray(inputs["down_wih"], f32)
    dn_whh = np.asarray(inputs["down_whh"], f32)
    obs_w = np.asarray(inputs["obs_w"], f32)
    out_w = np.asarray(inputs["out_w"], f32)

    lws = {}
    for pre, wih, whh in (("up", up_wih, up_whh), ("dn", dn_wih, dn_whh)):
        lws[f"{pre}_x_r"] = kron16(wih[0:2].T)
        lws[f"{pre}_x_z"] = kron16(wih[2:4].T)
        lws[f"{pre}_x_n"] = kron16(wih[4:6].T)
        lws[f"{pre}_h_r"] = kron16(whh[0:2].T)
        lws[f"{pre}_h_z"] = kron16(whh[2:4].T)
        lws[f"{pre}_h_n"] = kron16(whh[4:6].T)
    lws["obs01"] = kron16(obs_w[:, 0:2].T)
    lws["obs23"] = kron16(obs_w[:, 2:4].T)
    lws["obs4"] = kron16(obs_w[:, 4:5].T)
    lws["obsh"] = kron16(obs_w[:, 5:7].T)
    lws["out"] = kron16(out_w.T)
    lw_order = [
        "up_x_r", "up_x_z", "up_x_n", "up_h_r", "up_h_z", "up_h_n",
        "dn_x_r", "dn_x_z", "dn_x_n", "dn_h_r", "dn_h_z", "dn_h_n",
        "obs01", "obs23", "obs4", "obsh", "out",
    ]
    lwcat = np.zeros((2 * G, 2 * G * len(lw_order)), f16)
    for i, k in enumerate(lw_order):
        a = lws[k]
        lwcat[: a.shape[0], i * 2 * G: i * 2 * G + a.shape[1]] = a

    bcols = {}
    for pre, bih, bhh in (
        ("up", np.asarray(inputs["up_bih"], f32), np.asarray(inputs["up_bhh"], f32)),
        ("dn", np.asarray(inputs["down_bih"], f32), np.asarray(inputs["down_bhh"], f32)),
    ):
        bcols[f"{pre}_r"] = pcol(bih[0:2] + bhh[0:2])
        bcols[f"{pre}_z"] = pcol(bih[2:4] + bhh[2:4])
        bcols[f"{pre}_bhhn"] = pcol(bhh[4:6])
        bcols[f"{pre}_bihn"] = pcol(bih[4:6])
    bcols["obs"] = pcol(np.asarray(inputs["obs_b"], f32))
    bias_order = [
        "up_r", "up_z", "up_bhhn", "up_bihn",
        "dn_r", "dn_z", "dn_bhhn", "dn_bihn", "obs",
    ]
    biascat = np.concatenate([bcols[k] for k in bias_order], axis=1)
    return {"lwcat": lwcat, "biascat": np.ascontiguousarray(biascat)}


def kernel(**inputs) -> np.ndarray:
    from concourse.bass_utils import run_bass_kernel_spmd

    x = np.asarray(inputs["x"], np.float32)
    assert x.shape == (B, 19), x.shape

    if "nc" not in _CACHE:
        _CACHE["nc"] = _build_bass()
    nc = _CACHE["nc"]

    shared = _prepare_shared(inputs)
    in_maps = []
    for c in range(NCORES):
        xT_c = np.ascontiguousarray(x[c * BC:(c + 1) * BC].T).astype(np.float16)
        m = {"xT": xT_c}
        m.update(shared)
        in_maps.append(m)

    res = run_bass_kernel_spmd(nc, in_maps, list(range(NCORES)))

    y = np.empty((B, 7, 1), np.float32)
    for c in range(NCORES):
        y[c * BC:(c + 1) * BC, :, 0] = res.results[c]["yT"].T
    y += float(np.asarray(inputs["out_b"], np.float32).reshape(-1)[0])
    return y

